# revision 4
# baseline (speedup 1.0000x reference)
"""Cosine-similarity 1-NN over 1M x 256 f32 embeddings on 8 TRN2 NeuronCores.

Sharding: the stored-embedding table is split row-wise across the 8 cores
(cores 0-6 get contiguous 125952-row views; core 7 gets the remaining 118336
rows zero-padded to 125952 so all cores run the same SPMD graph).

Per-core graph (Bass/Tile):
  emb param [128, 984, 256] f32: partition p owns rows [p*984, (p+1)*984).
  Loop over 123 tiles of [128, 8*256]:
    - DMA tile from HBM
    - per 256-chunk: DVE tensor_tensor_reduce(e*q, add) -> dot column
    - per 256-chunk norms: ACT Square+accum (most tiles) or DVE TTR(e,e)
      (a few tiles, to balance the two engines' load)
  Epilogue: r = dot*|dot| / (||e||^2 + 1e-12)  (monotone in cosine sim),
  per-partition top-8 values + column indices via DVE max/max_index.
Host: rescores the 8*128*8 candidate rows exactly and picks the global best.
The true argmax is guaranteed to be among the candidates: within its
partition it has the highest r (sign-preserving monotone transform).
"""
import numpy as np
from contextlib import ExitStack

from concourse import bacc, tile, mybir
from concourse.bass_utils import run_bass_kernel_spmd

EPS = 1e-8
P = 128           # SBUF partitions
D = 256           # embedding dim
C = 984           # rows per partition per core
R = 8             # rows per partition per DMA tile
T = C // R        # tiles
ROWS_PC = P * C   # 125952 rows per core
N_CORES = 8
N_ROWS = 1000000

DVE_NORM_TILES = 13   # tiles whose norms run on DVE instead of ACT
DMA_SPLITS = 2
BUFS = 4


def _build():
    F = R * D
    nc = bacc.Bacc("TRN2", target_bir_lowering=False, debug=False,
                   num_devices=N_CORES)
    emb = nc.dram_tensor("emb", [P, C, D], mybir.dt.float32,
                         kind="ExternalInput").ap()
    q = nc.dram_tensor("q", [1, D], mybir.dt.float32, kind="ExternalInput").ap()
    out_r = nc.dram_tensor("out_r", [P, 8], mybir.dt.float32,
                           kind="ExternalOutput").ap()
    out_i = nc.dram_tensor("out_i", [P, 8], mybir.dt.uint32,
                           kind="ExternalOutput").ap()

    f32 = mybir.dt.float32
    Alu = mybir.AluOpType

    with tile.TileContext(nc) as tc:
        with ExitStack() as ctx:
            const_pool = ctx.enter_context(tc.tile_pool(name="const", bufs=1))
            emb_pool = ctx.enter_context(tc.tile_pool(name="emb", bufs=BUFS))
            scratch_pool = ctx.enter_context(tc.tile_pool(name="scr", bufs=2))
            act_scratch_pool = ctx.enter_context(tc.tile_pool(name="ascr", bufs=2))
            res_pool = ctx.enter_context(tc.tile_pool(name="res", bufs=1))

            qrep = const_pool.tile([P, D], f32)
            nc.sync.dma_start(out=qrep[:], in_=q.to_broadcast([P, D]))

            dots = res_pool.tile([P, C], f32)
            n2 = res_pool.tile([P, C], f32)

            for t in range(T):
                et = emb_pool.tile([P, F], f32, tag="embtile")
                rows_per_split = R // DMA_SPLITS
                for s in range(DMA_SPLITS):
                    r0 = s * rows_per_split
                    nc.sync.dma_start(
                        out=et[:, r0 * D:(r0 + rows_per_split) * D],
                        in_=emb[:, t * R + r0: t * R + r0 + rows_per_split, :])

                scr = scratch_pool.tile([P, F], f32, tag="dvescr")
                use_act = t < (T - DVE_NORM_TILES)
                if use_act:
                    ascr = act_scratch_pool.tile([P, F], f32, tag="actscr")
                for j in range(R):
                    c = t * R + j
                    sl = slice(j * D, (j + 1) * D)
                    nc.vector.scalar_tensor_tensor(
                        out=scr[:, sl], in0=et[:, sl], scalar=1.0,
                        in1=qrep[:], op0=Alu.mult, op1=Alu.mult,
                        accum_out=dots[:, c:c + 1])
                    if use_act:
                        nc.scalar.activation(
                            out=ascr[:, sl], in_=et[:, sl],
                            func=mybir.ActivationFunctionType.Square,
                            accum_out=n2[:, c:c + 1])
                    else:
                        nc.vector.scalar_tensor_tensor(
                            out=scr[:, sl], in0=et[:, sl], scalar=1.0,
                            in1=et[:, sl], op0=Alu.mult, op1=Alu.mult,
                            accum_out=n2[:, c:c + 1])

            ep = res_pool.tile([P, C], f32, tag="ep_absd")
            nc.scalar.activation(out=ep[:], in_=dots[:],
                                 func=mybir.ActivationFunctionType.Abs)
            nc.vector.tensor_mul(ep[:], ep[:], dots[:])
            n2e = res_pool.tile([P, C], f32, tag="ep_n2e")
            nc.vector.tensor_scalar(n2e[:], n2[:], 1e-12, scalar2=None,
                                    op0=Alu.add)
            n2r = res_pool.tile([P, C], f32, tag="ep_n2r")
            nc.vector.reciprocal(n2r[:], n2e[:])
            nc.vector.tensor_mul(ep[:], ep[:], n2r[:])

            rmax = res_pool.tile([P, 8], f32, tag="ep_rmax")
            ridx = res_pool.tile([P, 8], mybir.dt.uint32, tag="ep_ridx")
            nc.vector.max(out=rmax[:], in_=ep[:])
            nc.vector.max_index(out=ridx[:], in_max=rmax[:], in_values=ep[:])

            nc.sync.dma_start(out=out_r[:], in_=rmax[:])
            nc.sync.dma_start(out=out_i[:], in_=ridx[:])

    nc.compile()
    return nc


_NC_CACHE = None


def _get_nc():
    global _NC_CACHE
    if _NC_CACHE is None:
        _NC_CACHE = _build()
    return _NC_CACHE


def make_in_maps(query_embedding, stored_embeddings):
    q = np.asarray(query_embedding, dtype=np.float32)
    emb = np.ascontiguousarray(stored_embeddings, dtype=np.float32)
    qn = np.linalg.norm(q.astype(np.float64))
    qhat = (q.astype(np.float64) / (qn + EPS)).astype(np.float32).reshape(1, D)

    in_maps = []
    for i in range(N_CORES - 1):
        shard = emb[i * ROWS_PC:(i + 1) * ROWS_PC].reshape(P, C, D)
        in_maps.append({"emb": shard, "q": qhat})
    rest = emb[(N_CORES - 1) * ROWS_PC:]
    pad = np.zeros((ROWS_PC, D), dtype=np.float32)
    pad[:rest.shape[0]] = rest
    in_maps.append({"emb": pad.reshape(P, C, D), "q": qhat})
    return in_maps


def combine(results, query_embedding, stored_embeddings):
    """Pick the global best from per-core per-partition top-8 candidates."""
    q = np.asarray(query_embedding, dtype=np.float64)
    qhat = q / (np.linalg.norm(q) + EPS)
    cand = []
    for core, res in enumerate(results):
        idx = res["out_i"].astype(np.int64)          # [128, 8] column indices
        part = np.arange(P, dtype=np.int64)[:, None]
        g = core * ROWS_PC + part * C + idx          # global row ids
        cand.append(g.ravel())
    cand = np.concatenate(cand)
    cand = np.unique(cand[cand < N_ROWS])
    rows = np.asarray(stored_embeddings, dtype=np.float64)[cand]
    sims = (rows @ qhat) / (np.linalg.norm(rows, axis=1) + EPS)
    k = int(np.argmax(sims))
    best_idx = int(cand[k])
    best_score = np.float32(sims[k])
    return np.int32(best_idx), best_score


def kernel(query_embedding, stored_embeddings):
    nc = _get_nc()
    in_maps = make_in_maps(query_embedding, stored_embeddings)
    res = run_bass_kernel_spmd(nc, in_maps, core_ids=list(range(N_CORES)))
    return combine(res.results, query_embedding, stored_embeddings)


# revision 5
# speedup vs baseline: 1.3017x; 1.3017x over previous
"""Cosine-similarity 1-NN over 1M x 256 f32 embeddings on 8 TRN2 NeuronCores.

Sharding: the stored-embedding table is split row-wise across the 8 cores
(cores 0-6 get contiguous 125952-row views; core 7 gets the remaining 118336
rows zero-padded to 125952 so all cores run the same SPMD graph).

Per-core graph (Bass/Tile):
  emb param [128, 984, 256] f32: partition p owns rows [p*984, (p+1)*984).
  Loop over 41 tiles of [128, 24*256] (3 MB DMAs -> near HBM-peak bandwidth):
    per 256-chunk c:
    - DVE scalar_tensor_tensor(out=e*q, accum=dot) -> dots[:, c]
    - norms ||e||^2: ACT Square+accum_out for ~9/10 chunks, DVE STT(e,e) for
      the rest (balances the two engines; both land just under the DMA time)
  Epilogue: r = dot*|dot| / (||e||^2 + 1e-12)  (monotone in cosine sim),
  per-partition top-8 values + column indices via DVE max/max_index.
Host: rescores the 8*128*8 candidate rows exactly and picks the global best.
The true argmax is guaranteed to be among the candidates: within its
partition it has the highest r (sign-preserving monotone transform).
"""
import numpy as np
from contextlib import ExitStack

from concourse import bacc, tile, mybir
from concourse.bass_utils import run_bass_kernel_spmd

EPS = 1e-8
P = 128           # SBUF partitions
D = 256           # embedding dim
C = 984           # rows per partition per core
ROWS_PC = P * C   # 125952 rows per core
N_CORES = 8
N_ROWS = 1000000


def _build(C=C, R=24, dve_norm_every=10, bufs=4, num_devices=N_CORES,
           alternate_dma=True):
    assert C % R == 0
    T = C // R
    F = R * D
    nc = bacc.Bacc("TRN2", target_bir_lowering=False, debug=False,
                   num_devices=num_devices)
    emb = nc.dram_tensor("emb", [P, C, D], mybir.dt.float32,
                         kind="ExternalInput").ap()
    q = nc.dram_tensor("q", [1, D], mybir.dt.float32, kind="ExternalInput").ap()
    out_r = nc.dram_tensor("out_r", [P, 8], mybir.dt.float32,
                           kind="ExternalOutput").ap()
    out_i = nc.dram_tensor("out_i", [P, 8], mybir.dt.uint32,
                           kind="ExternalOutput").ap()

    f32 = mybir.dt.float32
    Alu = mybir.AluOpType

    with tile.TileContext(nc) as tc:
        with ExitStack() as ctx:
            const_pool = ctx.enter_context(tc.tile_pool(name="const", bufs=1))
            emb_pool = ctx.enter_context(tc.tile_pool(name="emb", bufs=bufs))
            scratch_pool = ctx.enter_context(tc.tile_pool(name="scr", bufs=1))
            res_pool = ctx.enter_context(tc.tile_pool(name="res", bufs=1))

            qrep = const_pool.tile([P, D], f32)
            nc.sync.dma_start(out=qrep[:], in_=q.to_broadcast([P, D]))

            dots = res_pool.tile([P, C], f32)
            n2 = res_pool.tile([P, C], f32)
            # single-slot scratches: written only by one engine, WAW-ordered
            scr = scratch_pool.tile([P, D], f32, tag="dvescr")
            ascr = scratch_pool.tile([P, D], f32, tag="actscr")

            nchunk = 0
            for t in range(T):
                et = emb_pool.tile([P, F], f32, tag="embtile")
                eng = nc.scalar if (alternate_dma and t % 2) else nc.sync
                eng.dma_start(out=et[:], in_=emb[:, t * R:(t + 1) * R, :])

                for j in range(R):
                    c = t * R + j
                    sl = slice(j * D, (j + 1) * D)
                    nc.vector.scalar_tensor_tensor(
                        out=scr[:], in0=et[:, sl], scalar=1.0,
                        in1=qrep[:], op0=Alu.mult, op1=Alu.mult,
                        accum_out=dots[:, c:c + 1])
                    nchunk += 1
                    if nchunk % dve_norm_every == 0:
                        nc.vector.scalar_tensor_tensor(
                            out=scr[:], in0=et[:, sl], scalar=1.0,
                            in1=et[:, sl], op0=Alu.mult, op1=Alu.mult,
                            accum_out=n2[:, c:c + 1])
                    else:
                        nc.scalar.activation(
                            out=ascr[:], in_=et[:, sl],
                            func=mybir.ActivationFunctionType.Square,
                            accum_out=n2[:, c:c + 1])

            # ---- epilogue: r = dot*|dot| / (n2 + tiny); top-8 per partition
            ep = res_pool.tile([P, C], f32, tag="ep_absd")
            nc.scalar.activation(out=ep[:], in_=dots[:],
                                 func=mybir.ActivationFunctionType.Abs)
            nc.vector.tensor_mul(ep[:], ep[:], dots[:])
            n2e = res_pool.tile([P, C], f32, tag="ep_n2e")
            nc.vector.tensor_scalar(n2e[:], n2[:], 1e-12, scalar2=None,
                                    op0=Alu.add)
            n2r = res_pool.tile([P, C], f32, tag="ep_n2r")
            nc.vector.reciprocal(n2r[:], n2e[:])
            nc.vector.tensor_mul(ep[:], ep[:], n2r[:])

            rmax = res_pool.tile([P, 8], f32, tag="ep_rmax")
            ridx = res_pool.tile([P, 8], mybir.dt.uint32, tag="ep_ridx")
            nc.vector.max(out=rmax[:], in_=ep[:])
            nc.vector.max_index(out=ridx[:], in_max=rmax[:], in_values=ep[:])

            nc.sync.dma_start(out=out_r[:], in_=rmax[:])
            nc.sync.dma_start(out=out_i[:], in_=ridx[:])

    nc.compile()
    return nc


_NC_CACHE = None


def _get_nc():
    global _NC_CACHE
    if _NC_CACHE is None:
        _NC_CACHE = _build()
    return _NC_CACHE


def make_in_maps(query_embedding, stored_embeddings):
    q = np.asarray(query_embedding, dtype=np.float32)
    emb = np.ascontiguousarray(stored_embeddings, dtype=np.float32)
    qn = np.linalg.norm(q.astype(np.float64))
    qhat = (q.astype(np.float64) / (qn + EPS)).astype(np.float32).reshape(1, D)

    in_maps = []
    for i in range(N_CORES - 1):
        shard = emb[i * ROWS_PC:(i + 1) * ROWS_PC].reshape(P, C, D)
        in_maps.append({"emb": shard, "q": qhat})
    rest = emb[(N_CORES - 1) * ROWS_PC:]
    pad = np.zeros((ROWS_PC, D), dtype=np.float32)
    pad[:rest.shape[0]] = rest
    in_maps.append({"emb": pad.reshape(P, C, D), "q": qhat})
    return in_maps


def combine(results, query_embedding, stored_embeddings):
    """Pick the global best from per-core per-partition top-8 candidates."""
    q = np.asarray(query_embedding, dtype=np.float64)
    qhat = q / (np.linalg.norm(q) + EPS)
    cand = []
    for core, res in enumerate(results):
        idx = res["out_i"].astype(np.int64)          # [128, 8] column indices
        part = np.arange(P, dtype=np.int64)[:, None]
        g = core * ROWS_PC + part * C + idx          # global row ids
        cand.append(g.ravel())
    cand = np.concatenate(cand)
    cand = np.unique(cand[cand < N_ROWS])
    rows = np.asarray(stored_embeddings, dtype=np.float64)[cand]
    sims = (rows @ qhat) / (np.linalg.norm(rows, axis=1) + EPS)
    k = int(np.argmax(sims))
    best_idx = int(cand[k])
    best_score = np.float32(sims[k])
    return np.int32(best_idx), best_score


def kernel(query_embedding, stored_embeddings):
    nc = _get_nc()
    in_maps = make_in_maps(query_embedding, stored_embeddings)
    res = run_bass_kernel_spmd(nc, in_maps, core_ids=list(range(N_CORES)))
    return combine(res.results, query_embedding, stored_embeddings)
